# revision 21
# baseline (speedup 1.0000x reference)
"""Multi-head attention (B=2, S=2048, D=1024, H=16, hd=64) with RoPE on 8 TRN2
NeuronCores.

Sharding: 2 batches x 4 head-groups. Core c handles batch c//4, heads
[4*(c%4), 4*(c%4)+4). Each core computes Q/K/V projections for its heads from
the full sequence, RoPE, unnormalized attention (exp(q.k/8) streamed through
PSUM with an exp(mask) column appended to V to collect the softmax row sums),
then normalizes. Loop order is q-tile outer / head-pair inner so that the
partial output projection for each 512-row q-tile completes mid-kernel and its
ReduceScatter (over the batch's 4-core group) overlaps the remaining
attention. The host reassembles the 4x4 (qtile, rank) x 128-row slices and
adds the (wo + wv@wo) bias.

Layout notes:
- x is uploaded pre-transposed (xT [D, S]) so it serves both as matmul rhs for
  Q^T/K^T production and as lhsT for V production.
- Q^T/K^T rows within each head are permuted to (d0,d32,d1,d33,...) so the
  RoPE partner lives in the adjacent partition; a stream_shuffle with the
  pair-swap mask plus two multiplies by host-precomputed cos/sin tables
  implements the rotation with all operands partition-aligned. The score
  matmul contracts over the permuted axis, which is permutation-invariant as
  long as Q and K share the ordering.
- The attention mask enters as exp(mask[k]) multiplied into V's rows (and
  the appended row-sum column), which is exact and free.
- Softmax row-sum reciprocals are folded via DMA into a [128, n] layout so
  the DVE divide runs on all lanes, then broadcast back via a DRAM round
  trip on the gpsimd queue.
"""

import numpy as np
import ml_dtypes

import concourse.bass as bass
import concourse.mybir as mybir
from concourse import bacc, bass_utils
import concourse.tile as tile

B, S, DIM, HEADS, HD = 2, 2048, 1024, 16, 64
HPC = HEADS // 4          # heads per core = 4
P = 128
NKC = DIM // P            # 8 contraction chunks for projections
NSC = S // P              # 16 sequence chunks of 128
NQT = S // 512            # 4 q tiles of 512
SQ = S // 4               # 512-row output slice per core
VW = HPC * (HD + 1)       # 260: V with a row-sum column per head
fp32 = mybir.dt.float32
bf16 = mybir.dt.bfloat16

_CACHE = {}


def _build(dbg=False):
    nc = bacc.Bacc("TRN2", target_bir_lowering=False, debug=False, num_devices=8)

    xT = nc.dram_tensor("xT", [DIM, S], bf16, kind="ExternalInput")
    wq = nc.dram_tensor("wq", [DIM, HPC * HD], bf16, kind="ExternalInput")
    wk = nc.dram_tensor("wk", [DIM, HPC * HD], bf16, kind="ExternalInput")
    wv = nc.dram_tensor("wv", [DIM, HPC * HD], bf16, kind="ExternalInput")
    wo = nc.dram_tensor("wo", [P, 8 * DIM], bf16, kind="ExternalInput")
    trigA = nc.dram_tensor("trigA", [P, S], bf16, kind="ExternalInput")
    trigB = nc.dram_tensor("trigB", [P, S], bf16, kind="ExternalInput")
    qbias = nc.dram_tensor("qbias", [P, 2], fp32, kind="ExternalInput")
    kbias = nc.dram_tensor("kbias", [P, 2], fp32, kind="ExternalInput")
    em = nc.dram_tensor("em", [P, NSC], fp32, kind="ExternalInput")
    out = nc.dram_tensor("out", [NQT * P, DIM], bf16, kind="ExternalOutput")

    SWAP_MASK = [i ^ 1 for i in range(32)]

    with tile.TileContext(nc) as tc:
        with (
            tc.tile_pool(name="const", bufs=1) as const,
            tc.tile_pool(name="work", bufs=3) as work,
            tc.tile_pool(name="attp", bufs=9) as attp,
            tc.tile_pool(name="stun", bufs=10) as stun,
            tc.tile_pool(name="ps_proj", bufs=2, space="PSUM") as ps_proj,
            tc.tile_pool(name="ps_o", bufs=2, space="PSUM") as ps_o,
            tc.tile_pool(name="ps_sT", bufs=2, space="PSUM") as ps_sT,
            tc.tile_pool(name="dram", bufs=1, space="DRAM") as dram,
            tc.tile_pool(name="dram_rc", bufs=3, space="DRAM") as dram_rc,
        ):
            # ---- load constants / inputs into SBUF ----
            # wq first (small, needed for the first projection matmul), then
            # xT chunks alternating between the sync and scalar HWDGE queues
            # so two uploads stream in parallel.
            wq_sb = const.tile([P, NKC, HPC * HD], bf16)
            nc.sync.dma_start(wq_sb[:], wq.rearrange("(c p) m -> p c m", p=P))
            wk_sb = const.tile([P, NKC, HPC * HD], bf16)
            nc.scalar.dma_start(wk_sb[:], wk.rearrange("(c p) m -> p c m", p=P))
            xT_sb = const.tile([P, NKC, S], bf16)
            xT_r = xT.rearrange("(c p) s -> p c s", p=P)
            for kc in range(NKC):
                eng = (nc.sync, nc.scalar, nc.gpsimd)[kc % 3]
                eng.dma_start(xT_sb[:, kc, :], xT_r[:, kc, :])
            qb_sb = const.tile([P, 2], fp32)
            nc.sync.dma_start(qb_sb[:], qbias[:])
            kb_sb = const.tile([P, 2], fp32)
            nc.sync.dma_start(kb_sb[:], kbias[:])
            em_sb = const.tile([P, NSC], fp32)
            nc.sync.dma_start(em_sb[:], em[:])
            wv_sb = const.tile([P, NKC, HPC * HD], bf16)
            nc.scalar.dma_start(wv_sb[:], wv.rearrange("(c p) m -> p c m", p=P))
            trigA_sb = const.tile([P, S], bf16)
            nc.sync.dma_start(trigA_sb[:], trigA[:])
            trigB_sb = const.tile([P, S], bf16)
            nc.scalar.dma_start(trigB_sb[:], trigB[:])
            wo_sb = const.tile([P, 8, DIM], bf16)
            nc.sync.dma_start(wo_sb[:], wo[:].rearrange("p (g m) -> p g m", g=8))

            warm_in = dram.tile([P, 4], fp32, name="warm_in")
            warm_out = dram.tile([P, 4], fp32, name="warm_out")
            wz = work.tile([P, 4], fp32, tag="wz", name="wz")
            nc.vector.memset(wz[:], 0.0)
            nc.gpsimd.dma_start(warm_in[:], wz[:])
            nc.gpsimd.collective_compute(
                "AllReduce", mybir.AluOpType.add,
                replica_groups=[[0, 1, 2, 3], [4, 5, 6, 7]],
                ins=[warm_in.opt()], outs=[warm_out.opt()],
            )
            wa_in = dram.tile([P, 4], fp32, name="wa_in")
            wa_out = dram.tile([8 * P, 4], fp32, name="wa_out")
            nc.gpsimd.dma_start(wa_in[:], wz[:])
            nc.gpsimd.collective_compute(
                "AllGather", mybir.AluOpType.bypass,
                replica_groups=[[0, 1, 2, 3, 4, 5, 6, 7]],
                ins=[wa_in.opt()], outs=[wa_out.opt()],
            )

            QT_rot = const.tile([P, 2, S], bf16)   # heads 0,1 | 2,3 stacked
            KT_rot = const.tile([P, 2, S], bf16)
            V_aug = const.tile([P, NSC, VW], bf16)  # [s-chunk, 4*(64+1)]

            # RoPE: bias add on ACT (idle in prelude) or DVE, then shuffle,
            # two trig muls, add on DVE.
            def rope_chain(pss_sc, b_sb, dst, cq, sc, on_scalar=True):
                q_sb = work.tile([P, 512], bf16, tag="q_sb",
                                 name=f"q_sb_{cq}_{sc}")
                if on_scalar:
                    nc.scalar.add(q_sb[:], pss_sc[:], b_sb[:, cq:cq + 1])
                else:
                    nc.vector.tensor_scalar_add(
                        q_sb[:], pss_sc[:], b_sb[:, cq:cq + 1])
                q_sw = work.tile([P, 512], bf16, tag="q_sw",
                                 name=f"q_sw_{cq}_{sc}")
                nc.vector.stream_shuffle(q_sw[:], q_sb[:], SWAP_MASK)
                p1 = work.tile([P, 512], bf16, tag="p1", name=f"p1_{cq}_{sc}")
                nc.vector.tensor_mul(
                    p1[:], q_sb[:], trigA_sb[:, sc * 512:(sc + 1) * 512])
                p2 = work.tile([P, 512], bf16, tag="p2", name=f"p2_{cq}_{sc}")
                nc.vector.tensor_mul(
                    p2[:], q_sw[:], trigB_sb[:, sc * 512:(sc + 1) * 512])
                nc.vector.tensor_add(
                    dst[:, cq, sc * 512:(sc + 1) * 512], p1[:], p2[:])

            # ---- Q^T cq0 / K^T cq0+cq1 projections + RoPE (prelude) ----
            # kc-outer so each xT chunk is consumed as soon as its DMA lands.
            # 4 sequence tiles accumulate at once: 2 from ps_proj, 2 from
            # ps_sT (borrowed; attention hasn't started).
            def proj_sc(w_sb, b_sb, dst, cq, sc, pool, tag):
                ps = pool.tile([P, 512], fp32, tag=tag,
                               name=f"pss_{cq}_{sc}_{dst is KT_rot}")
                for kc in range(NKC):
                    nc.tensor.matmul(
                        ps[:],
                        w_sb[:, kc, cq * P:(cq + 1) * P],
                        xT_sb[:, kc, sc * 512:(sc + 1) * 512],
                        start=(kc == 0), stop=(kc == NKC - 1),
                    )
                rope_chain(ps, b_sb, dst, cq, sc)

            # q/k tiles the first attention set needs come first; the rest
            # pipeline underneath attention via the dependency scheduler
            proj_sc(wq_sb, qb_sb, QT_rot, 0, 0, ps_proj, "proj")
            proj_sc(wk_sb, kb_sb, KT_rot, 0, 0, ps_sT, "sT")
            for sc in range(1, 4):
                proj_sc(wk_sb, kb_sb, KT_rot, 0, sc, ps_proj, "proj")
                proj_sc(wq_sb, qb_sb, QT_rot, 0, sc, ps_sT, "sT")
            for sc in range(4):
                pool, tag = (ps_proj, "proj") if sc % 2 == 0 else (ps_sT, "sT")
                proj_sc(wk_sb, kb_sb, KT_rot, 1, sc, pool, tag)

            # ---- V projection chunk (natural layout, scaled by exp(mask)) ----
            def v_chunk(sc):
                ps = ps_proj.tile([P, HPC * HD], fp32, tag="proj",
                                  name=f"vp_{sc}")
                for kc in range(NKC):
                    nc.tensor.matmul(
                        ps[:],
                        xT_sb[:, kc, sc * P:(sc + 1) * P],
                        wv_sb[:, kc, :],
                        start=(kc == 0), stop=(kc == NKC - 1),
                    )
                # per head: columns 0..63 = V * exp(mask), column 64 = exp(mask)
                vdst = V_aug[:, sc, :].rearrange("p (h x) -> p h x", h=HPC)
                nc.scalar.mul(
                    vdst[:, :, 0:HD],
                    ps[:].rearrange("p (h x) -> p h x", h=HPC),
                    em_sb[:, sc:sc + 1],
                )
                nc.vector.tensor_copy(
                    vdst[:, :, HD:HD + 1],
                    em_sb[:, sc:sc + 1, None].to_broadcast([P, HPC, 1]),
                )

            for sc in range(10):
                v_chunk(sc)

            # deferred work executed inside attention iterations:
            #  - remaining V chunks (10..15) early in (qt=0, hp=0); each is
            #    produced >=8 iterations before its attn_v consumer
            #  - Q cq=1 projection for tile qt during (qt, hp=0)
            defer_pss = {}

            def defer_v(it):
                if it < 6:
                    v_chunk(10 + it)

            def defer_q1(qt, m):
                # 8 matmuls (kc 0..7) over the first 8 iterations + rope
                if m < NKC:
                    kc = m
                    if kc == 0:
                        defer_pss[qt] = ps_proj.tile(
                            [P, 512], fp32, tag="proj", name=f"ip_{qt}")
                    nc.tensor.matmul(
                        defer_pss[qt][:],
                        wq_sb[:, kc, P:2 * P],
                        xT_sb[:, kc, qt * 512:(qt + 1) * 512],
                        start=(kc == 0), stop=(kc == NKC - 1),
                    )
                if m == NKC:
                    rope_chain(defer_pss[qt], qb_sb, QT_rot, 1, qt,
                               on_scalar=False)

            # ---- attention: q-tile outer, head pair inner ----
            oT_norm = const.tile([P, 2, S], bf16)   # normalized o^T, heads packed
            cc_ain = dram.tile([NQT, P, 2, 512], bf16, name="cc_ain")
            cc_aout = [dram.tile([8, P, 2, 512], bf16, name=f"cc_aout{q}",
                                 addr_space="Shared") for q in range(NQT)]
            a2a_sb = const.tile([P, 4, 2, 512], bf16)
            pid_sy = nc.sync.partition_id()
            pid_sc = nc.scalar.partition_id()
            rs_fold = const.tile([P, NQT, 2, 2, 4], bf16)  # row sums [qt, hp, j]

            def oproj(qt):
                # mirror this q-tile's normalized heads and gather it across
                # the 8 cores right away: core g only consumes chunk qt == g,
                # so its gather completes long before its own compute ends
                nc.sync.dma_start(
                    cc_ain[qt], oT_norm[:, :, qt * 512:(qt + 1) * 512])
                nc.gpsimd.collective_compute(
                    "AllGather", mybir.AluOpType.bypass,
                    replica_groups=[[0, 1, 2, 3, 4, 5, 6, 7]],
                    ins=[cc_ain[qt].opt()], outs=[cc_aout[qt].opt()],
                )
                for b_ in range(2):
                    for gp in range(4):
                        eng, pid = ((nc.sync, pid_sy) if gp % 2 == 0
                                    else (nc.scalar, pid_sc))
                        eng.dma_start(
                            a2a_sb[:, gp, :, :],
                            cc_aout[qt][4 * b_ + gp, :, :, :],
                            cond=(pid == 4 * b_ + qt))

            for qt in range(NQT):
                for hp in range(2):
                    oTs = [ps_o.tile([HD + 1, 512], fp32, tag="oT",
                                     name=f"oT_{qt}_{hp}_{j}")
                           for j in range(2)]

                    def attn_v(kb, at_kb, last=False):
                        for j in range(2):
                            h = 2 * hp + j
                            nc.tensor.matmul(
                                oTs[j][:],
                                V_aug[:, kb, h * (HD + 1):(h + 1) * (HD + 1)],
                                at_kb[:, j, :],
                                start=(kb == 0), stop=last,
                            )

                    # attnV software-pipelined two steps behind the exp
                    # stream; the slow DVE-poly tiles accumulate at the very
                    # end so they never block the in-order PE queue
                    pend, poly_pend = [], []
                    for kb in range(NSC):
                        sT = ps_sT.tile([P, 2, 512], fp32, tag="sT")
                        # the two heads occupy partitions 0-63 / 64-127, so
                        # the two K=64 score matmuls run concurrently
                        for j in range(2):
                            nc.tensor.matmul(
                                sT[:, j, :],
                                KT_rot[64 * j:64 * j + 64, hp, kb * P:(kb + 1) * P],
                                QT_rot[64 * j:64 * j + 64, hp,
                                       qt * 512:(qt + 1) * 512],
                                start=True, stop=True,
                            )
                        at = attp.tile([P, 2, 512], bf16, tag="attnT")
                        if kb in ():
                            # quartic exp(s/8) approximation on the DVE to
                            # offload the scalar engine (Horner, bf16)
                            A4, A3, A2, A1, A0 = (1.6111602240454273e-05,
                                                  0.0004326329486571948,
                                                  0.006825659218473084,
                                                  0.12019028625462389,
                                                  1.006537680536263)
                            x = work.tile([P, 2, 512], bf16, tag="expx",
                                          name=f"x_{qt}_{hp}_{kb}")
                            nc.vector.tensor_copy(x[:], sT[:])
                            h = work.tile([P, 2, 512], bf16, tag="exph",
                                          name=f"h_{qt}_{hp}_{kb}")
                            nc.vector.tensor_scalar(
                                h[:], x[:], A4, A3,
                                mybir.AluOpType.mult, mybir.AluOpType.add)
                            h2 = work.tile([P, 2, 512], bf16, tag="exph2",
                                           name=f"h2_{qt}_{hp}_{kb}")
                            nc.vector.tensor_mul(h2[:], h[:], x[:])
                            nc.vector.tensor_scalar_add(h[:], h2[:], A2)
                            nc.vector.tensor_mul(h2[:], h[:], x[:])
                            nc.vector.tensor_scalar_add(h[:], h2[:], A1)
                            nc.vector.tensor_mul(h2[:], h[:], x[:])
                            nc.vector.tensor_scalar_add(at[:], h2[:], A0)
                        else:
                            nc.scalar.activation(
                                at[:], sT[:],
                                mybir.ActivationFunctionType.Exp, scale=0.125)
                        if qt == 0:
                            defer_v(16 * hp + kb)
                        if hp == 0:
                            defer_q1(qt, kb)
                        if kb in ():
                            poly_pend.append((kb, at))
                        else:
                            pend.append((kb, at))
                        if len(pend) > 2:
                            attn_v(*pend.pop(0))
                    for p_ in pend:
                        attn_v(*p_)
                    for i_, p_ in enumerate(poly_pend):
                        attn_v(*p_, last=(i_ == len(poly_pend) - 1))

                    # stage the head values off PSUM (frees the banks) and
                    # fold the row sums into [128, 4] lanes via DMA
                    st_un = {}
                    for j in range(2):
                        st = stun.tile([HD + 1, 512], bf16, tag="st_un",
                                       name=f"st_un_{qt}_{hp}_{j}")
                        nc.vector.tensor_copy(st[:], oTs[j][:])
                        nc.sync.dma_start(
                            rs_fold[:, qt, hp, j, :], st[HD:HD + 1, :])
                        st_un[j] = st
                    # reciprocal on all 128 lanes at once, then broadcast
                    # back via DRAM on the gpsimd queue
                    rc = work.tile([P, 2, 4], bf16, tag="rc2",
                                   name=f"rc_{qt}_{hp}")
                    with nc.allow_low_precision(
                            reason="softmax scale in bf16 is within budget"):
                        nc.vector.reciprocal(rc[:], rs_fold[:, qt, hp, :, :])
                    rcd = dram_rc.tile([2, 512], bf16, tag="rcd",
                                       name=f"rcd_{qt}_{hp}")
                    for j in range(2):
                        nc.sync.dma_start(rcd[j:j + 1, :], rc[:, j, :])
                    for j in range(2):
                        pbase = 64 * j
                        rb = work.tile([HD, 512], bf16, tag="rbcast")
                        rsrc = rcd[j:j + 1, :]
                        nc.sync.dma_start(
                            rb[:],
                            bass.AP(rsrc.tensor, rsrc.offset, [[0, HD], [1, 512]]))
                        stage = work.tile([HD, 512], bf16, tag="stage")
                        nc.vector.tensor_mul(stage[:], st_un[j][0:HD, :], rb[:])
                        nc.sync.dma_start(
                            oT_norm[pbase:pbase + 64, hp,
                                    qt * 512:(qt + 1) * 512],
                            stage[:])
                oproj(qt)

            # ---- full O-projection from the gathered heads ----
            for qs in range(4):
                o_sb = work.tile([P, DIM], bf16, tag="o_sb", name=f"osb_{qs}")
                for dc in range(2):
                    ps = ps_proj.tile([P, 512], fp32, tag="proj",
                                      name=f"op_{qs}_{dc}")
                    for gp in range(4):
                        for c in range(2):
                            nc.tensor.matmul(
                                ps[:],
                                a2a_sb[:, gp, c, qs * P:(qs + 1) * P],
                                wo_sb[:, gp * 2 + c, dc * 512:(dc + 1) * 512],
                                start=(gp == 0 and c == 0),
                                stop=(gp == 3 and c == 1),
                            )
                    nc.vector.tensor_copy(
                        o_sb[:, dc * 512:(dc + 1) * 512], ps[:])
                nc.sync.dma_start(out[qs * P:(qs + 1) * P, :], o_sb[:])

    nc.compile()
    return nc


def _host_prep(x, pos, mask, wq_kernel, wq_bias, wk_kernel, wk_bias,
               wv_kernel, wv_bias, wo_kernel, wo_bias):
    """Build per-core in_maps for the 8 cores."""
    perm = np.array([(j // 2) if j % 2 == 0 else (j // 2 + 32)
                     for j in range(HD)])
    half = HD // 2
    freqs = (10000.0 ** (-np.linspace(0.0, 1.0, half, endpoint=False))).astype(np.float64)

    bf = ml_dtypes.bfloat16
    in_maps = []
    for c in range(8):
        b, g = c // 4, c % 4
        H = list(range(HPC * g, HPC * g + HPC))

        theta = pos[b].astype(np.float64)[:, None] * freqs[None, :]  # [S, 32]
        cos = np.cos(theta).astype(np.float32)
        sin = np.sin(theta).astype(np.float32)
        trigA = np.empty((P, S), np.float32)
        trigB = np.empty((P, S), np.float32)
        for r in range(P):
            j = r % HD
            i = j // 2
            trigA[r] = cos[:, i]
            trigB[r] = (-sin[:, i]) if j % 2 == 0 else sin[:, i]

        def permute_w(wk_):  # [D, H, hd] -> [D, 4*64] with rope-pair row order
            wsel = wk_[:, H, :][:, :, perm]          # [D, 4, 64]
            return np.ascontiguousarray(wsel.reshape(DIM, HPC * HD))

        def permute_b(bias):  # [H, hd] -> [128, 2]
            bsel = bias[H][:, perm]                  # [4, 64]
            return np.ascontiguousarray(bsel.reshape(2, P).T)

        emv = np.exp(mask[b, 0, 0].astype(np.float32))  # [S]

        in_maps.append({
            "xT": np.ascontiguousarray(x[b].T).astype(bf),
            "wq": permute_w(wq_kernel).astype(bf),
            "wk": permute_w(wk_kernel).astype(bf),
            "wv": np.ascontiguousarray(
                wv_kernel[:, H, :].reshape(DIM, HPC * HD)).astype(bf),
            "wo": np.ascontiguousarray(
                wo_kernel.reshape(4, 2, 2, HD, DIM)
                .transpose(2, 3, 0, 1, 4).reshape(P, 8 * DIM)).astype(bf),
            "trigA": trigA.astype(bf),
            "trigB": trigB.astype(bf),
            "qbias": permute_b(wq_bias),
            "kbias": permute_b(wk_bias),
            "em": np.ascontiguousarray(emv.reshape(NSC, P).T),
        })
    return in_maps


def kernel(x, pos, mask, wq_kernel, wq_bias, wk_kernel, wk_bias,
           wv_kernel, wv_bias, wo_kernel, wo_bias):
    x, pos, mask = np.asarray(x), np.asarray(pos), np.asarray(mask)
    wq_kernel, wq_bias = np.asarray(wq_kernel), np.asarray(wq_bias)
    wk_kernel, wk_bias = np.asarray(wk_kernel), np.asarray(wk_bias)
    wv_kernel, wv_bias = np.asarray(wv_kernel), np.asarray(wv_bias)
    wo_kernel, wo_bias = np.asarray(wo_kernel), np.asarray(wo_bias)
    if "nc" not in _CACHE:
        _CACHE["nc"] = _build()
    nc = _CACHE["nc"]

    in_maps = _host_prep(x, pos, mask, wq_kernel, wq_bias, wk_kernel, wk_bias,
                         wv_kernel, wv_bias, wo_kernel, wo_bias)
    res = bass_utils.run_bass_kernel_spmd(
        nc, in_maps, core_ids=list(range(8)))

    final_bias = (wo_bias.astype(np.float64)
                  + np.einsum("hd,hdo->o", wv_bias.astype(np.float64),
                              wo_kernel.astype(np.float64))).astype(np.float32)

    outs = []
    for b in range(B):
        rows = np.concatenate(
            [np.asarray(res.results[4 * b + g]["out"]).astype(np.float32)
             for g in range(4)], axis=0)
        outs.append(rows + final_bias[None, :])
    return np.stack(outs, axis=0)


# revision 22
# speedup vs baseline: 1.1024x; 1.1024x over previous
"""Multi-head attention (B=2, S=2048, D=1024, H=16, hd=64) with RoPE on 8 TRN2
NeuronCores.

Sharding: 2 batches x 4 head-groups. Core c handles batch c//4, heads
[4*(c%4), 4*(c%4)+4). Each core computes Q/K/V projections for its heads from
the full sequence, RoPE, unnormalized attention (exp(q.k/8) streamed through
PSUM with an exp(mask) column appended to V to collect the softmax row sums),
then normalizes. Loop order is q-tile outer / head-pair inner so that the
partial output projection for each 512-row q-tile completes mid-kernel and its
ReduceScatter (over the batch's 4-core group) overlaps the remaining
attention. The host reassembles the 4x4 (qtile, rank) x 128-row slices and
adds the (wo + wv@wo) bias.

Layout notes:
- x is uploaded pre-transposed (xT [D, S]) so it serves both as matmul rhs for
  Q^T/K^T production and as lhsT for V production.
- Q^T/K^T rows within each head are permuted to (d0,d32,d1,d33,...) so the
  RoPE partner lives in the adjacent partition; a stream_shuffle with the
  pair-swap mask plus two multiplies by host-precomputed cos/sin tables
  implements the rotation with all operands partition-aligned. The score
  matmul contracts over the permuted axis, which is permutation-invariant as
  long as Q and K share the ordering.
- The attention mask enters as exp(mask[k]) multiplied into V's rows (and
  the appended row-sum column), which is exact and free.
- Softmax row-sum reciprocals are folded via DMA into a [128, n] layout so
  the DVE divide runs on all lanes, then broadcast back via a DRAM round
  trip on the gpsimd queue.
"""

import numpy as np
import ml_dtypes

import concourse.bass as bass
import concourse.mybir as mybir
from concourse import bacc, bass_utils
import concourse.tile as tile

B, S, DIM, HEADS, HD = 2, 2048, 1024, 16, 64
HPC = HEADS // 4          # heads per core = 4
P = 128
NKC = DIM // P            # 8 contraction chunks for projections
NSC = S // P              # 16 sequence chunks of 128
NQT = S // 512            # 4 q tiles of 512
SQ = S // 4               # 512-row output slice per core
VW = HPC * (HD + 1)       # 260: V with a row-sum column per head
fp32 = mybir.dt.float32
bf16 = mybir.dt.bfloat16

_CACHE = {}


def _build(dbg=False):
    nc = bacc.Bacc("TRN2", target_bir_lowering=False, debug=False, num_devices=8)

    xT = nc.dram_tensor("xT", [DIM, S], bf16, kind="ExternalInput")
    wq = nc.dram_tensor("wq", [DIM, HPC * HD], bf16, kind="ExternalInput")
    wk = nc.dram_tensor("wk", [DIM, HPC * HD], bf16, kind="ExternalInput")
    wv = nc.dram_tensor("wv", [DIM, HPC * HD], bf16, kind="ExternalInput")
    wo = nc.dram_tensor("wo", [P, 8 * DIM], bf16, kind="ExternalInput")
    trigA = nc.dram_tensor("trigA", [P, S], bf16, kind="ExternalInput")
    trigB = nc.dram_tensor("trigB", [P, S], bf16, kind="ExternalInput")
    qbias = nc.dram_tensor("qbias", [P, 2], fp32, kind="ExternalInput")
    kbias = nc.dram_tensor("kbias", [P, 2], fp32, kind="ExternalInput")
    em = nc.dram_tensor("em", [P, NSC], fp32, kind="ExternalInput")
    out = nc.dram_tensor("out", [NQT * P, DIM], bf16, kind="ExternalOutput")

    SWAP_MASK = [i ^ 1 for i in range(32)]

    with tile.TileContext(nc) as tc:
        with (
            tc.tile_pool(name="const", bufs=1) as const,
            tc.tile_pool(name="work", bufs=3) as work,
            tc.tile_pool(name="attp", bufs=9) as attp,
            tc.tile_pool(name="stun", bufs=10) as stun,
            tc.tile_pool(name="ps_proj", bufs=2, space="PSUM") as ps_proj,
            tc.tile_pool(name="ps_o", bufs=2, space="PSUM") as ps_o,
            tc.tile_pool(name="ps_sT", bufs=2, space="PSUM") as ps_sT,
            tc.tile_pool(name="dram", bufs=1, space="DRAM") as dram,
            tc.tile_pool(name="dram_rc", bufs=3, space="DRAM") as dram_rc,
        ):
            # ---- load constants / inputs into SBUF ----
            # wq first (small, needed for the first projection matmul), then
            # xT chunks alternating between the sync and scalar HWDGE queues
            # so two uploads stream in parallel.
            wq_sb = const.tile([P, NKC, HPC * HD], bf16)
            nc.sync.dma_start(wq_sb[:], wq.rearrange("(c p) m -> p c m", p=P))
            wk_sb = const.tile([P, NKC, HPC * HD], bf16)
            nc.scalar.dma_start(wk_sb[:], wk.rearrange("(c p) m -> p c m", p=P))
            xT_sb = const.tile([P, NKC, S], bf16)
            xT_r = xT.rearrange("(c p) s -> p c s", p=P)
            for kc in range(NKC):
                eng = (nc.sync, nc.scalar, nc.gpsimd)[kc % 3]
                eng.dma_start(xT_sb[:, kc, :], xT_r[:, kc, :])
            qb_sb = const.tile([P, 2], fp32)
            nc.sync.dma_start(qb_sb[:], qbias[:])
            kb_sb = const.tile([P, 2], fp32)
            nc.sync.dma_start(kb_sb[:], kbias[:])
            em_sb = const.tile([P, NSC], fp32)
            nc.sync.dma_start(em_sb[:], em[:])
            wv_sb = const.tile([P, NKC, HPC * HD], bf16)
            nc.scalar.dma_start(wv_sb[:], wv.rearrange("(c p) m -> p c m", p=P))
            trigA_sb = const.tile([P, S], bf16)
            nc.sync.dma_start(trigA_sb[:], trigA[:])
            trigB_sb = const.tile([P, S], bf16)
            nc.scalar.dma_start(trigB_sb[:], trigB[:])
            wo_sb = const.tile([P, 8, DIM], bf16)
            nc.sync.dma_start(wo_sb[:], wo[:].rearrange("p (g m) -> p g m", g=8))

            warm_in = dram.tile([P, 4], fp32, name="warm_in")
            warm_out = dram.tile([P, 4], fp32, name="warm_out")
            wz = work.tile([P, 4], fp32, tag="wz", name="wz")
            nc.vector.memset(wz[:], 0.0)
            nc.gpsimd.dma_start(warm_in[:], wz[:])
            nc.gpsimd.collective_compute(
                "AllReduce", mybir.AluOpType.add,
                replica_groups=[[0, 1, 2, 3], [4, 5, 6, 7]],
                ins=[warm_in.opt()], outs=[warm_out.opt()],
            )
            wa_in = dram.tile([P, 4], fp32, name="wa_in")
            wa_out = dram.tile([8 * P, 4], fp32, name="wa_out")
            nc.gpsimd.dma_start(wa_in[:], wz[:])
            nc.gpsimd.collective_compute(
                "AllGather", mybir.AluOpType.bypass,
                replica_groups=[[0, 1, 2, 3, 4, 5, 6, 7]],
                ins=[wa_in.opt()], outs=[wa_out.opt()],
            )

            QT_rot = const.tile([P, 2, S], bf16)   # heads 0,1 | 2,3 stacked
            KT_rot = const.tile([P, 2, S], bf16)
            V_aug = const.tile([P, NSC, VW], bf16)  # [s-chunk, 4*(64+1)]

            # RoPE: bias add on ACT (idle in prelude) or DVE, then shuffle,
            # two trig muls, add on DVE.
            def rope_chain(pss_sc, b_sb, dst, cq, sc, on_scalar=True):
                q_sb = work.tile([P, 512], bf16, tag="q_sb",
                                 name=f"q_sb_{cq}_{sc}")
                if on_scalar:
                    nc.scalar.add(q_sb[:], pss_sc[:], b_sb[:, cq:cq + 1])
                else:
                    nc.vector.tensor_scalar_add(
                        q_sb[:], pss_sc[:], b_sb[:, cq:cq + 1])
                q_sw = work.tile([P, 512], bf16, tag="q_sw",
                                 name=f"q_sw_{cq}_{sc}")
                nc.vector.stream_shuffle(q_sw[:], q_sb[:], SWAP_MASK)
                p1 = work.tile([P, 512], bf16, tag="p1", name=f"p1_{cq}_{sc}")
                nc.vector.tensor_mul(
                    p1[:], q_sb[:], trigA_sb[:, sc * 512:(sc + 1) * 512])
                p2 = work.tile([P, 512], bf16, tag="p2", name=f"p2_{cq}_{sc}")
                nc.vector.tensor_mul(
                    p2[:], q_sw[:], trigB_sb[:, sc * 512:(sc + 1) * 512])
                nc.vector.tensor_add(
                    dst[:, cq, sc * 512:(sc + 1) * 512], p1[:], p2[:])

            # ---- Q^T cq0 / K^T cq0+cq1 projections + RoPE (prelude) ----
            # kc-outer so each xT chunk is consumed as soon as its DMA lands.
            # 4 sequence tiles accumulate at once: 2 from ps_proj, 2 from
            # ps_sT (borrowed; attention hasn't started).
            def proj_sc(w_sb, b_sb, dst, cq, sc, pool, tag):
                ps = pool.tile([P, 512], fp32, tag=tag,
                               name=f"pss_{cq}_{sc}_{dst is KT_rot}")
                for kc in range(NKC):
                    nc.tensor.matmul(
                        ps[:],
                        w_sb[:, kc, cq * P:(cq + 1) * P],
                        xT_sb[:, kc, sc * 512:(sc + 1) * 512],
                        start=(kc == 0), stop=(kc == NKC - 1),
                    )
                rope_chain(ps, b_sb, dst, cq, sc)

            # q/k tiles the first attention set needs come first; the rest
            # pipeline underneath attention via the dependency scheduler
            proj_sc(wq_sb, qb_sb, QT_rot, 0, 0, ps_proj, "proj")
            proj_sc(wk_sb, kb_sb, KT_rot, 0, 0, ps_sT, "sT")
            for sc in range(1, 4):
                proj_sc(wk_sb, kb_sb, KT_rot, 0, sc, ps_proj, "proj")
                proj_sc(wq_sb, qb_sb, QT_rot, 0, sc, ps_sT, "sT")
            for sc in range(4):
                pool, tag = (ps_proj, "proj") if sc % 2 == 0 else (ps_sT, "sT")
                proj_sc(wk_sb, kb_sb, KT_rot, 1, sc, pool, tag)

            # ---- V projection chunk (natural layout, scaled by exp(mask)) ----
            def v_chunk(sc):
                ps = ps_proj.tile([P, HPC * HD], fp32, tag="proj",
                                  name=f"vp_{sc}")
                for kc in range(NKC):
                    nc.tensor.matmul(
                        ps[:],
                        xT_sb[:, kc, sc * P:(sc + 1) * P],
                        wv_sb[:, kc, :],
                        start=(kc == 0), stop=(kc == NKC - 1),
                    )
                # per head: columns 0..63 = V * exp(mask), column 64 = exp(mask)
                vdst = V_aug[:, sc, :].rearrange("p (h x) -> p h x", h=HPC)
                nc.scalar.mul(
                    vdst[:, :, 0:HD],
                    ps[:].rearrange("p (h x) -> p h x", h=HPC),
                    em_sb[:, sc:sc + 1],
                )
                nc.vector.tensor_copy(
                    vdst[:, :, HD:HD + 1],
                    em_sb[:, sc:sc + 1, None].to_broadcast([P, HPC, 1]),
                )

            for sc in range(10):
                v_chunk(sc)

            # deferred work executed inside attention iterations:
            #  - remaining V chunks (10..15) early in (qt=0, hp=0); each is
            #    produced >=8 iterations before its attn_v consumer
            #  - Q cq=1 projection for tile qt during (qt, hp=0)
            defer_pss = {}

            def defer_v(it):
                if it < 6:
                    v_chunk(10 + it)

            def defer_q1(qt, m):
                # 8 matmuls (kc 0..7) over the first 8 iterations + rope
                if m < NKC:
                    kc = m
                    if kc == 0:
                        defer_pss[qt] = ps_proj.tile(
                            [P, 512], fp32, tag="proj", name=f"ip_{qt}")
                    nc.tensor.matmul(
                        defer_pss[qt][:],
                        wq_sb[:, kc, P:2 * P],
                        xT_sb[:, kc, qt * 512:(qt + 1) * 512],
                        start=(kc == 0), stop=(kc == NKC - 1),
                    )
                if m == NKC:
                    rope_chain(defer_pss[qt], qb_sb, QT_rot, 1, qt,
                               on_scalar=False)

            # ---- attention: q-tile outer, head pair inner ----
            oT_norm = const.tile([P, 2, S], bf16)   # normalized o^T, heads packed
            cc_ain = dram.tile([P, 2, S], bf16, name="cc_ain")
            cc_aout = dram.tile([8, P, 2, S], bf16, name="cc_aout",
                                addr_space="Shared")
            rs_fold = const.tile([P, NQT, 2, 2, 4], bf16)  # row sums [qt, hp, j]

            def oproj(qt):
                # mirror this q-tile's normalized heads to the gather buffer
                nc.sync.dma_start(
                    cc_ain[:, :, qt * 512:(qt + 1) * 512],
                    oT_norm[:, :, qt * 512:(qt + 1) * 512])

            for qt in range(NQT):
                for hp in range(2):
                    oTs = [ps_o.tile([HD + 1, 512], fp32, tag="oT",
                                     name=f"oT_{qt}_{hp}_{j}")
                           for j in range(2)]

                    def attn_v(kb, at_kb, last=False):
                        for j in range(2):
                            h = 2 * hp + j
                            nc.tensor.matmul(
                                oTs[j][:],
                                V_aug[:, kb, h * (HD + 1):(h + 1) * (HD + 1)],
                                at_kb[:, j, :],
                                start=(kb == 0), stop=last,
                            )

                    # attnV software-pipelined two steps behind the exp
                    # stream; the slow DVE-poly tiles accumulate at the very
                    # end so they never block the in-order PE queue
                    pend, poly_pend = [], []
                    for kb in range(NSC):
                        sT = ps_sT.tile([P, 2, 512], fp32, tag="sT")
                        # the two heads occupy partitions 0-63 / 64-127, so
                        # the two K=64 score matmuls run concurrently
                        for j in range(2):
                            nc.tensor.matmul(
                                sT[:, j, :],
                                KT_rot[64 * j:64 * j + 64, hp, kb * P:(kb + 1) * P],
                                QT_rot[64 * j:64 * j + 64, hp,
                                       qt * 512:(qt + 1) * 512],
                                start=True, stop=True,
                            )
                        at = attp.tile([P, 2, 512], bf16, tag="attnT")
                        if kb in ():
                            # quartic exp(s/8) approximation on the DVE to
                            # offload the scalar engine (Horner, bf16)
                            A4, A3, A2, A1, A0 = (1.6111602240454273e-05,
                                                  0.0004326329486571948,
                                                  0.006825659218473084,
                                                  0.12019028625462389,
                                                  1.006537680536263)
                            x = work.tile([P, 2, 512], bf16, tag="expx",
                                          name=f"x_{qt}_{hp}_{kb}")
                            nc.vector.tensor_copy(x[:], sT[:])
                            h = work.tile([P, 2, 512], bf16, tag="exph",
                                          name=f"h_{qt}_{hp}_{kb}")
                            nc.vector.tensor_scalar(
                                h[:], x[:], A4, A3,
                                mybir.AluOpType.mult, mybir.AluOpType.add)
                            h2 = work.tile([P, 2, 512], bf16, tag="exph2",
                                           name=f"h2_{qt}_{hp}_{kb}")
                            nc.vector.tensor_mul(h2[:], h[:], x[:])
                            nc.vector.tensor_scalar_add(h[:], h2[:], A2)
                            nc.vector.tensor_mul(h2[:], h[:], x[:])
                            nc.vector.tensor_scalar_add(h[:], h2[:], A1)
                            nc.vector.tensor_mul(h2[:], h[:], x[:])
                            nc.vector.tensor_scalar_add(at[:], h2[:], A0)
                        else:
                            nc.scalar.activation(
                                at[:], sT[:],
                                mybir.ActivationFunctionType.Exp, scale=0.125)
                        if qt == 0:
                            defer_v(16 * hp + kb)
                        if hp == 0:
                            defer_q1(qt, kb)
                        if kb in ():
                            poly_pend.append((kb, at))
                        else:
                            pend.append((kb, at))
                        if len(pend) > 2:
                            attn_v(*pend.pop(0))
                    for p_ in pend:
                        attn_v(*p_)
                    for i_, p_ in enumerate(poly_pend):
                        attn_v(*p_, last=(i_ == len(poly_pend) - 1))

                    # stage the head values off PSUM (frees the banks) and
                    # fold the row sums into [128, 4] lanes via DMA
                    st_un = {}
                    for j in range(2):
                        st = stun.tile([HD + 1, 512], bf16, tag="st_un",
                                       name=f"st_un_{qt}_{hp}_{j}")
                        nc.vector.tensor_copy(st[:], oTs[j][:])
                        nc.sync.dma_start(
                            rs_fold[:, qt, hp, j, :], st[HD:HD + 1, :])
                        st_un[j] = st
                    # reciprocal on all 128 lanes at once, then broadcast
                    # back via DRAM on the gpsimd queue
                    rc = work.tile([P, 2, 4], bf16, tag="rc2",
                                   name=f"rc_{qt}_{hp}")
                    with nc.allow_low_precision(
                            reason="softmax scale in bf16 is within budget"):
                        nc.vector.reciprocal(rc[:], rs_fold[:, qt, hp, :, :])
                    rcd = dram_rc.tile([2, 512], bf16, tag="rcd",
                                       name=f"rcd_{qt}_{hp}")
                    for j in range(2):
                        nc.sync.dma_start(rcd[j:j + 1, :], rc[:, j, :])
                    for j in range(2):
                        pbase = 64 * j
                        rb = work.tile([HD, 512], bf16, tag="rbcast")
                        rsrc = rcd[j:j + 1, :]
                        nc.sync.dma_start(
                            rb[:],
                            bass.AP(rsrc.tensor, rsrc.offset, [[0, HD], [1, 512]]))
                        stage = work.tile([HD, 512], bf16, tag="stage")
                        nc.vector.tensor_mul(stage[:], st_un[j][0:HD, :], rb[:])
                        nc.sync.dma_start(
                            oT_norm[pbase:pbase + 64, hp,
                                    qt * 512:(qt + 1) * 512],
                            stage[:])
                oproj(qt)

            # ---- exchange q-slices for head-groups, then full O-proj ----
            nc.gpsimd.collective_compute(
                "AllGather", mybir.AluOpType.bypass,
                replica_groups=[[0, 1, 2, 3, 4, 5, 6, 7]],
                ins=[cc_ain.opt()], outs=[cc_aout.opt()],
            )
            # gather my q-slice (rows [512g, 512g+512)) of my batch group's
            # four chunks via partition-id-derived offsets
            a2a_sb = const.tile([P, 4, 2, 512], bf16)
            CH = P * 2 * S
            engs = (nc.sync, nc.scalar, nc.gpsimd, nc.sync)
            pids = {e: e.partition_id() for e in set(engs)}
            for gp in range(4):
                eng = engs[gp]
                pid = pids[eng]
                off = ((pid // 4) * 4 + gp) * CH + (pid % 4) * 512
                src_ap = bass.AP(cc_aout.tensor, cc_aout.offset + off,
                                 [[2 * S, P], [S, 2], [1, 512]])
                eng.dma_start(a2a_sb[:, gp, :, :], src_ap)
            for qs in range(4):
                o_sb = work.tile([P, DIM], bf16, tag="o_sb", name=f"osb_{qs}")
                for dc in range(2):
                    ps = ps_proj.tile([P, 512], fp32, tag="proj",
                                      name=f"op_{qs}_{dc}")
                    for gp in range(4):
                        for c in range(2):
                            nc.tensor.matmul(
                                ps[:],
                                a2a_sb[:, gp, c, qs * P:(qs + 1) * P],
                                wo_sb[:, gp * 2 + c, dc * 512:(dc + 1) * 512],
                                start=(gp == 0 and c == 0),
                                stop=(gp == 3 and c == 1),
                            )
                    nc.vector.tensor_copy(
                        o_sb[:, dc * 512:(dc + 1) * 512], ps[:])
                nc.sync.dma_start(out[qs * P:(qs + 1) * P, :], o_sb[:])

    nc.compile()
    return nc


def _host_prep(x, pos, mask, wq_kernel, wq_bias, wk_kernel, wk_bias,
               wv_kernel, wv_bias, wo_kernel, wo_bias):
    """Build per-core in_maps for the 8 cores."""
    perm = np.array([(j // 2) if j % 2 == 0 else (j // 2 + 32)
                     for j in range(HD)])
    half = HD // 2
    freqs = (10000.0 ** (-np.linspace(0.0, 1.0, half, endpoint=False))).astype(np.float64)

    bf = ml_dtypes.bfloat16
    in_maps = []
    for c in range(8):
        b, g = c // 4, c % 4
        H = list(range(HPC * g, HPC * g + HPC))

        theta = pos[b].astype(np.float64)[:, None] * freqs[None, :]  # [S, 32]
        cos = np.cos(theta).astype(np.float32)
        sin = np.sin(theta).astype(np.float32)
        trigA = np.empty((P, S), np.float32)
        trigB = np.empty((P, S), np.float32)
        for r in range(P):
            j = r % HD
            i = j // 2
            trigA[r] = cos[:, i]
            trigB[r] = (-sin[:, i]) if j % 2 == 0 else sin[:, i]

        def permute_w(wk_):  # [D, H, hd] -> [D, 4*64] with rope-pair row order
            wsel = wk_[:, H, :][:, :, perm]          # [D, 4, 64]
            return np.ascontiguousarray(wsel.reshape(DIM, HPC * HD))

        def permute_b(bias):  # [H, hd] -> [128, 2]
            bsel = bias[H][:, perm]                  # [4, 64]
            return np.ascontiguousarray(bsel.reshape(2, P).T)

        emv = np.exp(mask[b, 0, 0].astype(np.float32))  # [S]

        in_maps.append({
            "xT": np.ascontiguousarray(x[b].T).astype(bf),
            "wq": permute_w(wq_kernel).astype(bf),
            "wk": permute_w(wk_kernel).astype(bf),
            "wv": np.ascontiguousarray(
                wv_kernel[:, H, :].reshape(DIM, HPC * HD)).astype(bf),
            "wo": np.ascontiguousarray(
                wo_kernel.reshape(4, 2, 2, HD, DIM)
                .transpose(2, 3, 0, 1, 4).reshape(P, 8 * DIM)).astype(bf),
            "trigA": trigA.astype(bf),
            "trigB": trigB.astype(bf),
            "qbias": permute_b(wq_bias),
            "kbias": permute_b(wk_bias),
            "em": np.ascontiguousarray(emv.reshape(NSC, P).T),
        })
    return in_maps


def kernel(x, pos, mask, wq_kernel, wq_bias, wk_kernel, wk_bias,
           wv_kernel, wv_bias, wo_kernel, wo_bias):
    x, pos, mask = np.asarray(x), np.asarray(pos), np.asarray(mask)
    wq_kernel, wq_bias = np.asarray(wq_kernel), np.asarray(wq_bias)
    wk_kernel, wk_bias = np.asarray(wk_kernel), np.asarray(wk_bias)
    wv_kernel, wv_bias = np.asarray(wv_kernel), np.asarray(wv_bias)
    wo_kernel, wo_bias = np.asarray(wo_kernel), np.asarray(wo_bias)
    if "nc" not in _CACHE:
        _CACHE["nc"] = _build()
    nc = _CACHE["nc"]

    in_maps = _host_prep(x, pos, mask, wq_kernel, wq_bias, wk_kernel, wk_bias,
                         wv_kernel, wv_bias, wo_kernel, wo_bias)
    res = bass_utils.run_bass_kernel_spmd(
        nc, in_maps, core_ids=list(range(8)))

    final_bias = (wo_bias.astype(np.float64)
                  + np.einsum("hd,hdo->o", wv_bias.astype(np.float64),
                              wo_kernel.astype(np.float64))).astype(np.float32)

    outs = []
    for b in range(B):
        rows = np.concatenate(
            [np.asarray(res.results[4 * b + g]["out"]).astype(np.float32)
             for g in range(4)], axis=0)
        outs.append(rows + final_bias[None, :])
    return np.stack(outs, axis=0)
